# revision 1
# baseline (speedup 1.0000x reference)
"""TRN2 Bass kernel for nn_DebateModel (v1 hybrid).

Device (8 NeuronCores, data-parallel over comments, 8 comments/core):
streams the full token_embed (the memory-dominant input, 201 MB) through
the bidirectional span-encoder input projections
    xp_d = W_ih_d @ x^T   for d in {fwd, bwd}   (fp16 operands, fp32 psum)
which is the bulk of the model's FLOPs and memory traffic.

Host: the sequential LSTM recurrences (latency-bound on TRN2's engines),
span gathers, the per-comment GAT/attention head and the comment
compressor, in fp32 numpy, consuming the device-computed projections.

Self-contained: hardcodes all shapes; no sibling imports.
"""
import sys
import numpy as np

sys.path.insert(0, '/opt/trn_rl_repo')

C, L, FEAT = 64, 1024, 768
H = 80
SPAN = 4 * H            # 320
GATES = 4 * H           # 320 per direction
N_CORES = 8
CPC = C // N_CORES      # comments per core = 8
TOK = CPC * L           # tokens per core = 8192
KCH = FEAT // 128       # 6 contraction chunks
GCH = (2 * GATES) // 128  # 5 gate chunks over both directions (640)
TBLK = 512              # moving-operand token block
NTB = TOK // TBLK       # 16

_compiled = None


def _build():
    import concourse.bass as bass
    import concourse.tile as tile
    from concourse import bacc, mybir
    from contextlib import ExitStack

    f16, f32 = mybir.dt.float16, mybir.dt.float32

    nc = bacc.Bacc("TRN2", target_bir_lowering=False, debug=False,
                   enable_asserts=False, num_devices=N_CORES)

    xt_d = nc.dram_tensor("xt", [KCH, 128, TOK], f16, kind="ExternalInput").ap()
    w_d = nc.dram_tensor("w", [KCH, GCH, 128, 128], f16,
                         kind="ExternalInput").ap()
    xp_d = nc.dram_tensor("xp", [GCH, 128, TOK], f32,
                          kind="ExternalOutput").ap()

    with tile.TileContext(nc) as tc, ExitStack() as ctx:
        wpool = ctx.enter_context(tc.tile_pool(name="w", bufs=1))
        xpool = ctx.enter_context(tc.tile_pool(name="x", bufs=3))
        opool = ctx.enter_context(tc.tile_pool(name="o", bufs=3))
        ppool = ctx.enter_context(tc.tile_pool(name="p", bufs=2, space="PSUM"))

        wt = wpool.tile([128, KCH * GCH * 128], f16)
        wt3 = {}
        for k in range(KCH):
            for g in range(GCH):
                wt3[k, g] = wt[:, bass.ts(k * GCH + g, 128)]
                nc.sync.dma_start(wt3[k, g], w_d[k, g])

        for tb in range(NTB):
            xts = []
            for k in range(KCH):
                xtile = xpool.tile([128, TBLK], f16, tag=f"x{k}")
                nc.sync.dma_start(xtile[:], xt_d[k, :, bass.ts(tb, TBLK)])
                xts.append(xtile)
            for g in range(GCH):
                ps = ppool.tile([128, TBLK], f32, tag="ps")
                for k in range(KCH):
                    nc.tensor.matmul(ps[:], wt3[k, g], xts[k][:],
                                     start=(k == 0), stop=(k == KCH - 1))
                ot = opool.tile([128, TBLK], f32, tag="ot")
                nc.scalar.copy(ot[:], ps[:])
                nc.sync.dma_start(xp_d[g, :, bass.ts(tb, TBLK)], ot[:])
    nc.compile()
    return nc


def _sigmoid(z):
    out = np.empty_like(z)
    np.negative(z, out)
    np.exp(out, out)
    out += 1.0
    np.reciprocal(out, out)
    return out


def _lstm(xp, Whh, b, reverse=False):
    """xp: [L, B, 320] precomputed x @ Wih.T. Returns hidden states
    [L, B, 80] fp32, exact fp32 serial recurrence."""
    Ln, B, _ = xp.shape
    Wt = Whh.T.astype(np.float32)
    h = np.zeros((B, H), np.float32)
    c = np.zeros((B, H), np.float32)
    hs = np.empty((Ln, B, H), np.float32)
    order = range(Ln - 1, -1, -1) if reverse else range(Ln)
    for t in order:
        z = xp[t] + h @ Wt + b
        i, f, g, o = z[:, :H], z[:, H:2*H], z[:, 2*H:3*H], z[:, 3*H:]
        c = _sigmoid(f) * c + _sigmoid(i) * np.tanh(g)
        h = _sigmoid(o) * np.tanh(c)
        hs[t] = h
    return hs


def _attn_pool(feats, vals, mask, W1, b1, W2, b2):
    s = np.maximum(feats @ W1 + b1, 0.0) @ W2 + b2
    s = np.where(mask[:, None], s, -1e9)
    ex = np.exp(s - s.max(0, keepdims=True))
    a = ex / ex.sum(0, keepdims=True)
    a = np.where(mask[:, None], a, 0.0)
    out = (a * vals).sum(0)
    return np.where(mask.any(), out, np.zeros_like(out))


def _gat(h, src, dst, emask, Wm, a_l, a_r, bias):
    An, K = h.shape[0], Wm.shape[0]
    hp = np.stack([h @ Wm[k] for k in range(K)], 1)          # [A, K, D]
    el = (hp * a_l[None]).sum(-1)
    er = (hp * a_r[None]).sum(-1)
    e = el[src] + er[dst]
    e = np.where(e > 0, e, 0.2 * e)
    e = np.where(emask[:, None], e, -1e9)
    m = np.full((An, K), -1e9, np.float32)
    np.maximum.at(m, dst, e)
    ex = np.where(emask[:, None], np.exp(e - m[dst]), 0.0)
    den = np.zeros((An, K), np.float32)
    np.add.at(den, dst, ex)
    alpha = ex / np.maximum(den[dst], 1e-9)
    out = np.zeros((An, K, hp.shape[2]), np.float32)
    np.add.at(out, dst, alpha[:, :, None] * hp[src])
    out = out + bias[None]
    out = np.where(out > 0, out, np.expm1(np.minimum(out, 0.0)))
    return out.reshape(An, -1)


def kernel(**inputs):
    global _compiled
    inp = {k: np.asarray(v) for k, v in inputs.items()}

    # ---- device: input projections over all tokens ----
    token = inp['token_embed'].astype(np.float32)            # [C, L, 768]
    Wih2 = np.concatenate([inp['Wih_f'], inp['Wih_b']], 0)   # [640, 768]
    # pack stationary chunks: w[k, g, p, q] = Wih2[g*128+q, k*128+p]
    wpk = np.ascontiguousarray(
        Wih2.reshape(GCH, 128, KCH, 128).transpose(2, 0, 3, 1)
    ).astype(np.float16)                                     # [6, 5, 128, 128]

    in_maps = []
    for core in range(N_CORES):
        tk = token[core*CPC:(core+1)*CPC]                     # [8, 1024, 768]
        xt = np.ascontiguousarray(
            tk.reshape(TOK, KCH, 128).transpose(1, 2, 0)
        ).astype(np.float16)                                  # [6, 128, 8192]
        in_maps.append(dict(xt=xt, w=wpk))

    if _compiled is None:
        _compiled = _build()
    globals()['_last_in_maps'] = in_maps
    from concourse.bass_utils import run_bass_kernel_spmd
    import time as _time
    _t0 = _time.time()
    res = run_bass_kernel_spmd(_compiled, in_maps,
                               core_ids=list(range(N_CORES)))
    globals()['_last_exec_ns'] = res.exec_time_ns
    globals()['_last_dispatch_s'] = _time.time() - _t0

    xp_all = np.empty((C, L, 2 * GATES), np.float32)
    for core in range(N_CORES):
        xpc = res.results[core]["xp"]                         # [5, 128, 8192]
        xpc = xpc.reshape(2 * GATES, CPC, L).transpose(1, 2, 0)
        xp_all[core*CPC:(core+1)*CPC] = xpc

    # ---- host: recurrences + heads (fp32) ----
    xp_f = np.ascontiguousarray(
        xp_all[:, :, :GATES].transpose(1, 0, 2)) + inp['b_f']  # [L, C, 320]
    xp_b = np.ascontiguousarray(
        xp_all[:, :, GATES:].transpose(1, 0, 2)) + inp['b_b']
    hf = _lstm(xp_f, inp['Whh_f'], 0.0).transpose(1, 0, 2)     # [C, L, 80]
    hb = _lstm(xp_b, inp['Whh_b'], 0.0, reverse=True).transpose(1, 0, 2)

    A = inp['adu_spans'].shape[1]
    W_gat = inp['W_gat'].astype(np.float32)

    def span_rep(c, spans):
        i, j = spans[..., 0], spans[..., 1]
        return np.concatenate([hf[c][j] - hf[c][i - 1], hb[c][i] - hb[c][j + 1],
                               hf[c][i - 1], hb[c][j + 1]], -1)

    rows = []
    for c in range(C):
        cemb = span_rep(c, inp['comment_spans'][c])
        amask = inp['adu_masks'][c]
        adus = span_rep(c, inp['adu_spans'][c]) * amask[:, None]
        isrc, idst = inp['inner_src'][c], inp['inner_dst'][c]
        irel, imask = inp['inner_rel'][c], inp['inner_mask'][c]
        tsrc, tdst = inp['inter_src'][c], inp['inter_dst'][c]
        trel, tmask = inp['inter_rel'][c], inp['inter_mask'][c]
        srcs = [isrc, isrc, tdst, tdst]
        dsts = [idst, idst, tsrc, tsrc]
        masks = [imask & (irel == 0), imask & (irel == 1),
                 tmask & (trel == 0), tmask & (trel == 1)]
        z = np.stack([_gat(adus, srcs[m], dsts[m], masks[m], W_gat[m],
                           inp['a_l'][m], inp['a_r'][m], inp['b_gat'][m])
                      for m in range(4)])                     # [4, A, 768]
        w = np.tanh(z.reshape(4 * A, -1) @ inp['W_sem'] + inp['b_sem'])
        w = (w @ inp['q_sem']).reshape(4, A)
        w = (w * amask[None]).sum(1) / max(amask.sum(), 1)
        beta = np.exp(w - w.max())
        beta /= beta.sum()
        zfin = np.einsum('m,mad->ad', beta, z)
        adu_embeds = zfin @ inp['W_pred'] + inp['b_pred']
        feats = np.concatenate(
            [np.broadcast_to(cemb, (A, SPAN)), adu_embeds], -1)
        att_adu = _attn_pool(feats, adu_embeds, amask & inp['local_masks'][c],
                             inp['W_adu1'], inp['b_adu1'],
                             inp['W_adu2'], inp['b_adu2'])

        def pair(se, de, rel, me, W1, b1, W2, b2):
            onehot = np.stack([rel, 1 - rel], -1).astype(np.float32)
            pe = np.concatenate([adu_embeds[se], adu_embeds[de], onehot], -1)
            fp = np.concatenate(
                [np.broadcast_to(cemb, (pe.shape[0], SPAN)), pe], -1)
            return _attn_pool(fp, pe, me, W1, b1, W2, b2)

        att_inn = pair(isrc, idst, irel, imask, inp['W_inn1'], inp['b_inn1'],
                       inp['W_inn2'], inp['b_inn2'])
        att_int = pair(tdst, tsrc, trel, tmask, inp['W_int1'], inp['b_int1'],
                       inp['W_int2'], inp['b_int2'])
        rows.append(np.concatenate(
            [att_adu, att_inn, att_int, inp['info_scores'][c], cemb]))
    wo_ctx = np.stack(rows).astype(np.float32)                # [64, 1608]

    xpc = (wo_ctx @ inp['Wih_c'].T + inp['b_c'])[:, None, :]  # [64, 1, 800]
    globals()['H'], hs = 200, None
    try:
        hs = _lstm(xpc, inp['Whh_c'], 0.0)[:, 0, :]           # [64, 200]
    finally:
        globals()['H'] = 80
    return np.concatenate([hs, wo_ctx], -1).astype(np.float32)



# revision 15
# speedup vs baseline: 21.5869x; 21.5869x over previous
"""TRN2 Bass kernel for nn_DebateModel (v2: on-device BiLSTM).

Device (8 NeuronCores, data-parallel over comments, 8 comments/core):
  Stage A: input projections xp[d,G] = Wih_{d,G} @ x^T + b for both LSTM
           directions (fp16 operands, fp32 psum), resident in SBUF.
  Stage B: the full bidirectional LSTM recurrence (1024 steps, fwd+bwd
           interleaved, c in fp32, h in fp16), emitting hidden states
           h[80, t, dir, comment] to DRAM as fp16.
Only the hidden states (2.6 MB/core) leave the device, instead of the
168 MB of raw gate projections the v1 hybrid shipped back.

Host: span gathers, per-comment GAT/attention heads and the comment
compressor LSTM, in fp32 numpy (all tiny).

Self-contained: hardcodes all shapes; no sibling imports.
"""
import sys
import numpy as np

sys.path.insert(0, '/opt/trn_rl_repo')

C, L, FEAT = 64, 1024, 768
H = 80
SPAN = 4 * H            # 320
N_CORES = 8
CPC = C // N_CORES      # comments per core = 8
TOK = CPC * L           # tokens per core = 8192
KCH = FEAT // 128       # 6 contraction chunks
TBLK = 512              # moving-operand token block (one half-comment)
NTB = TOK // TBLK       # 16
# gate-group order per direction: i, f, o, g (sigmoid, sigmoid, sigmoid, tanh)
GSLICE = [(0, 80), (80, 160), (240, 320), (160, 240)]

_compiled = None


def _build():
    import concourse.bass as bass
    import concourse.tile as tile
    from concourse import bacc, mybir
    from concourse.bass import ds
    from contextlib import ExitStack

    f16, f32 = mybir.dt.float16, mybir.dt.float32
    SIG = mybir.ActivationFunctionType.Sigmoid
    TANH = mybir.ActivationFunctionType.Tanh
    IDENT = mybir.ActivationFunctionType.Identity
    BYP = mybir.AluOpType.bypass
    ADD = mybir.AluOpType.add
    UB = 128                 # steps per loop iteration

    nc = bacc.Bacc("TRN2", target_bir_lowering=False, debug=False,
                   enable_asserts=False, num_devices=N_CORES)

    xt_d = nc.dram_tensor("xt", [KCH, 128, TOK], f16, kind="ExternalInput").ap()
    wih_d = nc.dram_tensor("wih", [128, 8, KCH, H], f16,
                           kind="ExternalInput").ap()
    whh_d = nc.dram_tensor("whh", [H, 8, H], f16, kind="ExternalInput").ap()
    bias_d = nc.dram_tensor("bias", [H, 8], f32, kind="ExternalInput").ap()
    hout_d = nc.dram_tensor("hout", [H, L, 2, CPC], f16,
                            kind="ExternalOutput").ap()

    with tile.TileContext(nc) as tc, ExitStack() as ctx:
        rpool = ctx.enter_context(tc.tile_pool(name="res", bufs=1))
        # xp laid out t-major over (gate, comment) so a 128-step block is
        # one contiguous 8KB run per partition
        xp_sb = rpool.tile([H, 2, L, 4, CPC], f16)
        wih_sb = rpool.tile([128, 8, KCH, H], f16)
        whh_sb = rpool.tile([H, 8, H], f16)
        bias_sb = rpool.tile([H, 8], f32)
        cinit = rpool.tile([H, 2, CPC], f32)
        hstg = rpool.tile([H, UB, 2, CPC], f16)
        xpw_f = rpool.tile([H, UB, 4, CPC], f16)
        xpw_b = rpool.tile([H, UB, 4, CPC], f16)
        nc.sync.dma_start(wih_sb[:], wih_d[:])
        nc.sync.dma_start(whh_sb[:], whh_d[:])
        nc.sync.dma_start(bias_sb[:], bias_d[:])
        nc.vector.memset(cinit[:], 0.0)
        nc.vector.memset(hstg[:, UB - 1, :, :], 0.0)

        # ---- Stage A: xp[d,t,G,c] = Wih @ x + b, resident fp16 ----
        with tc.tile_pool(name="xs", bufs=2) as xpool, \
             tc.tile_pool(name="pA", bufs=2, space="PSUM") as ppa:
            for tb in range(NTB):
                cid, t0 = tb // 2, (tb % 2) * TBLK
                xts = []
                for k in range(KCH):
                    xtile = xpool.tile([128, TBLK], f16, tag=f"x{k}")
                    nc.sync.dma_start(xtile[:],
                                      xt_d[k, :, tb * TBLK:(tb + 1) * TBLK])
                    xts.append(xtile)
                for g in range(8):
                    d, gi = g // 4, g % 4
                    ps = ppa.tile([H, TBLK], f32, tag="ps")
                    for k in range(KCH):
                        nc.tensor.matmul(ps[:], wih_sb[:, g, k, :], xts[k][:],
                                         start=(k == 0), stop=(k == KCH - 1))
                    nc.scalar.activation(
                        xp_sb[:, d, t0:t0 + TBLK, gi, cid], ps[:], IDENT,
                        bias=bias_sb[:, g:g + 1])

        # ---- Stage B: LSTM recurrence, fwd (t=s) + bwd (t=1023-s) ----
        # One dynamic loop, UB unrolled steps per iteration. The bwd
        # direction's xp block is prefetched from the mirrored time range
        # and indexed in reverse, so all per-step APs are static.
        with tc.tile_pool(name="sB", bufs=4) as sp, \
             tc.tile_pool(name="pB", bufs=4, space="PSUM") as ppb:
            with tc.For_i(0, L, UB) as iv:
                nc.sync.dma_start(xpw_f[:], xp_sb[:, 0, ds(iv, UB), :, :])
                nc.sync.dma_start(xpw_b[:],
                                  xp_sb[:, 1, ds(L - UB - iv, UB), :, :])
                hf_ap = hstg[:, UB - 1, 0, :]
                hb_ap = hstg[:, UB - 1, 1, :]
                c_prev = cinit
                for j in range(UB):
                    z = ppb.tile([H, 2, 4, CPC], f32, tag="z")
                    for g in range(8):
                        d, gi = g // 4, g % 4
                        nc.tensor.matmul(z[:, d, gi, :], whh_sb[:, g, :],
                                         hf_ap if d == 0 else hb_ap,
                                         start=True, stop=True)
                    zs = sp.tile([H, 2, 4, CPC], f32, tag="zs")
                    nc.vector.scalar_tensor_tensor(
                        zs[:, 0, :, :], z[:, 0, :, :], 0.0,
                        xpw_f[:, j, :, :], BYP, ADD)
                    nc.vector.scalar_tensor_tensor(
                        zs[:, 1, :, :], z[:, 1, :, :], 0.0,
                        xpw_b[:, UB - 1 - j, :, :], BYP, ADD)
                    za = sp.tile([H, 2, 4, CPC], f32, tag="za")
                    nc.scalar.activation(za[:, :, 0:3, :], zs[:, :, 0:3, :],
                                         SIG)
                    nc.scalar.activation(za[:, :, 3, :], zs[:, :, 3, :], TANH)
                    t1 = sp.tile([H, 2, CPC], f32, tag="t1")
                    t2 = sp.tile([H, 2, CPC], f32, tag="t2")
                    nc.vector.tensor_mul(t1[:], za[:, :, 0, :], za[:, :, 3, :])
                    nc.vector.tensor_mul(t2[:], za[:, :, 1, :], c_prev[:])
                    c_cur = sp.tile([H, 2, CPC], f32, tag="c")
                    nc.vector.tensor_add(c_cur[:], t1[:], t2[:])
                    th = sp.tile([H, 2, CPC], f32, tag="th")
                    nc.scalar.activation(th[:], c_cur[:], TANH)
                    nc.vector.tensor_mul(hstg[:, j, :, :], za[:, :, 2, :],
                                         th[:])
                    hf_ap, hb_ap = hstg[:, j, 0, :], hstg[:, j, 1, :]
                    c_prev = c_cur
                # carry c into the fixed tile the next iteration reads
                nc.vector.tensor_copy(cinit[:], c_prev[:])
                nc.sync.dma_start(hout_d[:, ds(iv, UB), :, :], hstg[:])
    nc.compile()
    return nc


def _sigmoid(z):
    out = np.empty_like(z)
    np.negative(z, out)
    np.exp(out, out)
    out += 1.0
    np.reciprocal(out, out)
    return out


def _lstm200(xp, Whh):
    """Comment-compressor LSTM: xp [T, 800] precomputed x @ Wih.T + b."""
    Hc = 200
    Wt = Whh.T.astype(np.float32)
    h = np.zeros(Hc, np.float32)
    c = np.zeros(Hc, np.float32)
    hs = np.empty((xp.shape[0], Hc), np.float32)
    for t in range(xp.shape[0]):
        zt = xp[t] + h @ Wt
        i, f, g, o = zt[:Hc], zt[Hc:2*Hc], zt[2*Hc:3*Hc], zt[3*Hc:]
        c = _sigmoid(f) * c + _sigmoid(i) * np.tanh(g)
        h = _sigmoid(o) * np.tanh(c)
        hs[t] = h
    return hs


def _attn_pool(feats, vals, mask, W1, b1, W2, b2):
    s = np.maximum(feats @ W1 + b1, 0.0) @ W2 + b2
    s = np.where(mask[:, None], s, -1e9)
    ex = np.exp(s - s.max(0, keepdims=True))
    a = ex / ex.sum(0, keepdims=True)
    a = np.where(mask[:, None], a, 0.0)
    out = (a * vals).sum(0)
    return np.where(mask.any(), out, np.zeros_like(out))


def _gat(h, src, dst, emask, Wm, a_l, a_r, bias):
    An, K = h.shape[0], Wm.shape[0]
    hp = np.stack([h @ Wm[k] for k in range(K)], 1)          # [A, K, D]
    el = (hp * a_l[None]).sum(-1)
    er = (hp * a_r[None]).sum(-1)
    e = el[src] + er[dst]
    e = np.where(e > 0, e, 0.2 * e)
    e = np.where(emask[:, None], e, -1e9)
    m = np.full((An, K), -1e9, np.float32)
    np.maximum.at(m, dst, e)
    ex = np.where(emask[:, None], np.exp(e - m[dst]), 0.0)
    den = np.zeros((An, K), np.float32)
    np.add.at(den, dst, ex)
    alpha = ex / np.maximum(den[dst], 1e-9)
    out = np.zeros((An, K, hp.shape[2]), np.float32)
    np.add.at(out, dst, alpha[:, :, None] * hp[src])
    out = out + bias[None]
    out = np.where(out > 0, out, np.expm1(np.minimum(out, 0.0)))
    return out.reshape(An, -1)


def kernel(**inputs):
    global _compiled
    inp = {k: np.asarray(v) for k, v in inputs.items()}

    # ---- pack device inputs ----
    token = inp['token_embed'].astype(np.float32)            # [C, L, 768]
    Wih_f, Wih_b = inp['Wih_f'], inp['Wih_b']
    Whh_f, Whh_b = inp['Whh_f'], inp['Whh_b']
    b_f, b_b = inp['b_f'], inp['b_b']
    # gate-group stack order: fwd i,f,o,g then bwd i,f,o,g
    Wg = np.stack([Wih_f[a:b] for a, b in GSLICE]
                  + [Wih_b[a:b] for a, b in GSLICE])         # [8, 80, 768]
    wih_pk = np.ascontiguousarray(
        Wg.reshape(8, H, KCH, 128).transpose(3, 0, 2, 1)).astype(np.float16)
    Wh = np.stack([Whh_f[a:b] for a, b in GSLICE]
                  + [Whh_b[a:b] for a, b in GSLICE])         # [8, 80, 80]
    whh_pk = np.ascontiguousarray(Wh.transpose(2, 0, 1)).astype(np.float16)
    bs = np.stack([b_f[a:b] for a, b in GSLICE]
                  + [b_b[a:b] for a, b in GSLICE])           # [8, 80]
    bias_pk = np.ascontiguousarray(bs.T).astype(np.float32)  # [80, 8]

    in_maps = []
    for core in range(N_CORES):
        tk = token[core*CPC:(core+1)*CPC]                     # [8, 1024, 768]
        xt = np.ascontiguousarray(
            tk.reshape(TOK, KCH, 128).transpose(1, 2, 0)
        ).astype(np.float16)                                  # [6, 128, 8192]
        in_maps.append(dict(xt=xt, wih=wih_pk, whh=whh_pk, bias=bias_pk))

    if _compiled is None:
        _compiled = _build()
    globals()['_last_in_maps'] = in_maps
    from concourse.bass_utils import run_bass_kernel_spmd
    import time as _time
    _t0 = _time.time()
    res = run_bass_kernel_spmd(_compiled, in_maps,
                               core_ids=list(range(N_CORES)))
    globals()['_last_exec_ns'] = res.exec_time_ns
    globals()['_last_dispatch_s'] = _time.time() - _t0

    # hout [80, 1024, 2, 8]: fwd at [:, t, 0, c]; bwd at [:, 1023-t, 1, c]
    hf = np.empty((C, L, H), np.float32)
    hb = np.empty((C, L, H), np.float32)
    for core in range(N_CORES):
        ho = res.results[core]["hout"].astype(np.float32)     # [80,1024,2,8]
        hf[core*CPC:(core+1)*CPC] = ho[:, :, 0, :].transpose(2, 1, 0)
        hb[core*CPC:(core+1)*CPC] = ho[::, ::-1, 1, :].transpose(2, 1, 0)

    # ---- host: span gathers + heads (fp32) ----
    A = inp['adu_spans'].shape[1]
    W_gat = inp['W_gat'].astype(np.float32)

    def span_rep(c, spans):
        i, j = spans[..., 0], spans[..., 1]
        return np.concatenate([hf[c][j] - hf[c][i - 1], hb[c][i] - hb[c][j + 1],
                               hf[c][i - 1], hb[c][j + 1]], -1)

    rows = []
    for c in range(C):
        cemb = span_rep(c, inp['comment_spans'][c])
        amask = inp['adu_masks'][c]
        adus = span_rep(c, inp['adu_spans'][c]) * amask[:, None]
        isrc, idst = inp['inner_src'][c], inp['inner_dst'][c]
        irel, imask = inp['inner_rel'][c], inp['inner_mask'][c]
        tsrc, tdst = inp['inter_src'][c], inp['inter_dst'][c]
        trel, tmask = inp['inter_rel'][c], inp['inter_mask'][c]
        srcs = [isrc, isrc, tdst, tdst]
        dsts = [idst, idst, tsrc, tsrc]
        masks = [imask & (irel == 0), imask & (irel == 1),
                 tmask & (trel == 0), tmask & (trel == 1)]
        z = np.stack([_gat(adus, srcs[m], dsts[m], masks[m], W_gat[m],
                           inp['a_l'][m], inp['a_r'][m], inp['b_gat'][m])
                      for m in range(4)])                     # [4, A, 768]
        w = np.tanh(z.reshape(4 * A, -1) @ inp['W_sem'] + inp['b_sem'])
        w = (w @ inp['q_sem']).reshape(4, A)
        w = (w * amask[None]).sum(1) / max(amask.sum(), 1)
        beta = np.exp(w - w.max())
        beta /= beta.sum()
        zfin = np.einsum('m,mad->ad', beta, z)
        adu_embeds = zfin @ inp['W_pred'] + inp['b_pred']
        feats = np.concatenate(
            [np.broadcast_to(cemb, (A, SPAN)), adu_embeds], -1)
        att_adu = _attn_pool(feats, adu_embeds, amask & inp['local_masks'][c],
                             inp['W_adu1'], inp['b_adu1'],
                             inp['W_adu2'], inp['b_adu2'])

        def pair(se, de, rel, me, W1, b1, W2, b2):
            onehot = np.stack([rel, 1 - rel], -1).astype(np.float32)
            pe = np.concatenate([adu_embeds[se], adu_embeds[de], onehot], -1)
            fp = np.concatenate(
                [np.broadcast_to(cemb, (pe.shape[0], SPAN)), pe], -1)
            return _attn_pool(fp, pe, me, W1, b1, W2, b2)

        att_inn = pair(isrc, idst, irel, imask, inp['W_inn1'], inp['b_inn1'],
                       inp['W_inn2'], inp['b_inn2'])
        att_int = pair(tdst, tsrc, trel, tmask, inp['W_int1'], inp['b_int1'],
                       inp['W_int2'], inp['b_int2'])
        rows.append(np.concatenate(
            [att_adu, att_inn, att_int, inp['info_scores'][c], cemb]))
    wo_ctx = np.stack(rows).astype(np.float32)                # [64, 1608]

    xpc = wo_ctx @ inp['Wih_c'].T + inp['b_c']                # [64, 800]
    hs = _lstm200(xpc, inp['Whh_c'])                          # [64, 200]
    return np.concatenate([hs, wo_ctx], -1).astype(np.float32)
